# revision 10
# baseline (speedup 1.0000x reference)
"""Trainium2 Bass kernel for the ConvolutionalKAN problem.

Math: the KAN conv
    out[b,o,y,x] = sum_{j,kk,l,m} phi_m(11*inp[b,j,y+kk,x+l]) * coeff[o,j,kk,l,m]
with phi_m the degree-3 B-spline basis on uniform knots linspace(0,1,12).
With u = |11x-(m+2)|:  6*phi_m = relu(2-u)^3 - 4*relu(1-u)^3, so folding
coeff/6 into the weights makes this a VALID 3x3 conv over 64*8 = 512 input
channels, computed here entirely in fp16 (sim rel-err ~9e-4, limit 2e-2).

Basis pipeline (8 passes per q-tile of 2 basis fns x 64 cin, all fp16):
    u   = Abs(11x + b)          ACT
    sb  = Square(1 - u)         ACT     (unclamped; clamp comes from bm)
    am  = min(u-2, 0) = -relu(2-u)   DVE tensor_scalar
    bm  = min(u-1, 0) = -relu(1-u)   Pool tensor_scalar
    sa  = am*am                 DVE
    a3m = sa*am = -relu(2-u)^3  DVE
    b3m = sb*bm = -relu(1-u)^3  DVE
    g   = 4*b3m - a3m = 6*phi   DVE scalar_tensor_tensor

PE ("L-pack", M=128): x-taps l=0,1 are packed as the 128 lhsT columns
(full PE array) accumulating psum_AB[(l,o), y, xs] over raw 64-wide rows
(N=512/bank); l=2 runs half-width chains with two output groups sharing one
PSUM bank via tile_position. Column shifts are resolved by two shifted
post-adds straight out of PSUM (Pool + DVE):
    out[o,y,x] = psAB[o,y,x] + psAB[64+o,y,x+1] + psC[o,y,x+2]
This needs ~24.4 PE cycles/output pixel vs 36 for the M=64 windowed form.

Sharding: data-parallel over batch, 2 images per core on 8 cores.
"""

import os
import sys

import numpy as np

for _p in ("/root/.axon_site/_ro/trn_rl_repo", "/opt/trn_rl_repo"):
    if os.path.isdir(_p) and _p not in sys.path:
        sys.path.append(_p)

B_FULL = 16
N_CORES = 8
B_SHARD = B_FULL // N_CORES
CIN = 64
COUT = 64
H = 64
W = 64
KS = 3
NB = 8
HO = H - KS + 1  # 62
WO = W - KS + 1  # 62
NQ = (CIN * NB) // 128  # 4 contraction tiles of 128
NPIX = H * W

# input-row chunks for the elementwise basis pipeline (aligned so chunk 0
# exactly covers the rows needed by the first two output groups)
CHUNKS = [(0, 18), (18, 16), (34, 16), (50, 14)]
# output row groups (y0, n_rows)
GROUPS = [(0, 8), (8, 8), (16, 8), (24, 8), (32, 8), (40, 8), (48, 8), (56, 6)]
# pairs of groups sharing one C-chain PSUM bank
WAVES = [(0, 1), (2, 3), (4, 5), (6, 7)]

SB_ENG = os.environ.get("KAN_SB_ENG", "scalar")   # engine for sb square
SA_ENG = os.environ.get("KAN_SA_ENG", "vector")   # engine for sa square
BM_ENG = os.environ.get("KAN_BM_ENG", "gpsimd")   # engine for bm rail
# NOTE: GPSIMD/Pool cannot access PSUM, and no instruction may read two
# PSUM operands; so psum post-processing is an ACT copy + one DVE add.
CP_ENG = os.environ.get("KAN_CP_ENG", "scalar")
ADD_ENG = os.environ.get("KAN_ADD_ENG", "vector")


def _fold_coeff(coeff: np.ndarray) -> np.ndarray:
    """coeff [COUT, CIN, KS, KS, NB] -> W_host [128, NQ*KS*192] fp16.

    Column layout per (q, kk) block of 192: [l=0 | l=1] form the 128-wide
    AB pair, [l=2] the 64-wide C block. Partition k = (m - 2q)*64 + cin.
    """
    w6 = coeff.astype(np.float64) / 6.0  # [o, c, kk, l, m]
    arr = np.transpose(w6, (4, 1, 2, 3, 0))  # [m, c, kk, l, o]
    blocks = []
    for q in range(NQ):
        blk = arr[2 * q:2 * q + 2].reshape(128, KS, KS * COUT)  # [k, kk, l*o]
        blocks.append(blk.reshape(128, KS * KS * COUT))
    whost = np.concatenate(blocks, axis=1)  # [128, NQ*576]
    return np.ascontiguousarray(whost, dtype=np.float16)


def _build_bass():
    import concourse.bacc as bacc
    import concourse.mybir as mybir
    import concourse.tile as tile

    f32 = mybir.dt.float32
    f16 = mybir.dt.float16
    AF = mybir.ActivationFunctionType
    alu = mybir.AluOpType

    nc = bacc.Bacc("TRN2", target_bir_lowering=False, debug=False,
                   num_devices=N_CORES)
    x_d = nc.dram_tensor("x", [B_SHARD, CIN, H, W], f32, kind="ExternalInput").ap()
    w_d = nc.dram_tensor("w", [128, NQ * KS * KS * COUT], f16,
                         kind="ExternalInput").ap()
    b_d = nc.dram_tensor("btbl", [128, NQ], f32, kind="ExternalInput").ap()
    out_d = nc.dram_tensor("out", [B_SHARD, COUT, HO, WO], f32,
                           kind="ExternalOutput").ap()

    with tile.TileContext(nc) as tc:
        from contextlib import ExitStack

        with ExitStack() as ctx:
            cpool = ctx.enter_context(tc.tile_pool(name="const", bufs=1))
            wpool = ctx.enter_context(tc.tile_pool(name="w", bufs=1))
            xpool = ctx.enter_context(tc.tile_pool(name="x", bufs=2))
            gpool = ctx.enter_context(tc.tile_pool(name="g", bufs=2 * NQ))
            rpool = ctx.enter_context(tc.tile_pool(name="r", bufs=3))
            opool = ctx.enter_context(tc.tile_pool(name="o", bufs=4))
            pabpool = ctx.enter_context(
                tc.tile_pool(name="pab", bufs=6, space="PSUM"))

            bt = cpool.tile([128, NQ], f32)
            nc.sync.dma_start(bt[:], b_d[:])
            wc = wpool.tile([128, NQ * KS * KS * COUT], f16)
            nc.sync.dma_start(wc[:], w_d[:])

            def eng(name):
                return {"scalar": nc.scalar, "vector": nc.vector,
                        "gpsimd": nc.gpsimd}[name]

            for b in range(B_SHARD):
                xt = xpool.tile([128, NPIX], f32)
                src = x_d[b, :, :, :]
                nc.sync.dma_start(
                    xt[0:64].rearrange("p (r c) -> p r c", c=W), src)
                nc.sync.dma_start(
                    xt[64:128].rearrange("p (r c) -> p r c", c=W), src)

                gts = []
                for q in range(NQ):
                    gts.append(gpool.tile([128, NPIX], f16, tag="g",
                                          name=f"g{b}_{q}"))

                # elementwise basis, chunked by input-row ranges
                for (r0, nr_in) in CHUNKS:
                    npx = nr_in * W
                    lo, hi = r0 * W, r0 * W + npx
                    for q in range(NQ):
                        u = rpool.tile([128, 18 * W], f16, tag="u")
                        sb = rpool.tile([128, 18 * W], f16, tag="sb")
                        am = rpool.tile([128, 18 * W], f16, tag="am")
                        bm = rpool.tile([128, 18 * W], f16, tag="bm")
                        sa = rpool.tile([128, 18 * W], f16, tag="sa")
                        a3m = rpool.tile([128, 18 * W], f16, tag="a3m")
                        b3m = rpool.tile([128, 18 * W], f16, tag="b3m")
                        nc.scalar.activation(u[:, :npx], xt[:, lo:hi],
                                             AF.Abs, bias=bt[:, q:q + 1],
                                             scale=11.0)
                        if SB_ENG == "scalar":
                            nc.scalar.activation(sb[:, :npx], u[:, :npx],
                                                 AF.Square, bias=1.0,
                                                 scale=-1.0)
                        else:
                            # sb = bm*bm (needs bm first; emitted below)
                            pass
                        nc.vector.tensor_scalar(am[:, :npx], u[:, :npx],
                                                2.0, 0.0, alu.subtract,
                                                alu.min)
                        eng(BM_ENG).tensor_scalar(bm[:, :npx], u[:, :npx],
                                                  1.0, 0.0, alu.subtract,
                                                  alu.min)
                        if SB_ENG != "scalar":
                            eng(SB_ENG).tensor_mul(sb[:, :npx], bm[:, :npx],
                                                   bm[:, :npx])
                        if SA_ENG == "scalar":
                            nc.scalar.activation(sa[:, :npx], u[:, :npx],
                                                 AF.Square, bias=2.0,
                                                 scale=-1.0)
                        else:
                            eng(SA_ENG).tensor_mul(sa[:, :npx], am[:, :npx],
                                                   am[:, :npx])
                        nc.vector.tensor_mul(a3m[:, :npx], sa[:, :npx],
                                             am[:, :npx])
                        nc.vector.tensor_mul(b3m[:, :npx], sb[:, :npx],
                                             bm[:, :npx])
                        nc.vector.scalar_tensor_tensor(
                            gts[q][:, lo:hi], b3m[:, :npx], 4.0,
                            a3m[:, :npx], op0=alu.mult, op1=alu.subtract)

                gvs = [g[:].rearrange("p (r c) -> p r c", c=W) for g in gts]

                # matmuls: per wave of 2 groups, one PSUM bank per group.
                # Bank layout: partitions 0:64 accumulate l=0 (raw rows) and
                # l=2 (rhs column-shifted by 2, M=64); partitions 64:128
                # accumulate l=1 (raw rows, paired in the same lhsT as l=0).
                n_mm = NQ * KS
                for (ga, gb) in WAVES:
                    wave = (ga, gb)
                    psab = {g: pabpool.tile([128, 8, W], f32, tag="psab",
                                            name=f"psab{g}")
                            for g in wave}
                    i_mm = 0
                    for q in range(NQ):
                        for kk in range(KS):
                            base = q * (KS * KS * COUT) + kk * (KS * COUT)
                            wab = wc[:, base:base + 2 * COUT]
                            wcc = wc[:, base + 2 * COUT:base + 3 * COUT]
                            first = i_mm == 0
                            last = i_mm == n_mm - 1
                            for grp in wave:
                                y0, nr = GROUPS[grp]
                                rhs = gvs[q][:, y0 + kk:y0 + kk + nr, :]
                                rhs2 = gvs[q][:, y0 + kk:y0 + kk + nr, 2:W]
                                if first:
                                    nc.tensor.matmul(psab[grp][:, :nr, :],
                                                     wab, rhs, start=True,
                                                     stop=False)
                                    nc.tensor.matmul(
                                        psab[grp][0:64, :nr, 0:W - 2], wcc,
                                        rhs2, start=False, stop=False)
                                else:
                                    nc.tensor.matmul(
                                        psab[grp][0:64, :nr, 0:W - 2], wcc,
                                        rhs2, start=False, stop=last)
                                    nc.tensor.matmul(psab[grp][:, :nr, :],
                                                     wab, rhs, start=False,
                                                     stop=last)
                            i_mm += 1
                    for grp in wave:
                        y0, nr = GROUPS[grp]
                        t = opool.tile([64, 8, WO], f32, tag="t")
                        if CP_ENG == "scalar":
                            nc.scalar.activation(
                                t[:, :nr, :], psab[grp][64:128, :nr, 1:1 + WO],
                                AF.Identity, bias=0.0, scale=1.0)
                        else:
                            eng(CP_ENG).tensor_copy(
                                t[:, :nr, :], psab[grp][64:128, :nr, 1:1 + WO])
                        ot = opool.tile([64, 8, WO], f32, tag="ot")
                        eng(ADD_ENG).tensor_tensor(
                            ot[:, :nr, :], t[:, :nr, :],
                            psab[grp][0:64, :nr, 0:WO], alu.add)
                        nc.sync.dma_start(out_d[b, :, y0:y0 + nr, :],
                                          ot[:, :nr, :])

    nc.compile()
    return nc


def _maybe_install_profile_shim():
    """Allow trace=True/BASS_TRACE under axon even though this image lacks
    antenv.axon_hooks; degrade silently if anything is missing."""
    import types

    if "antenv.axon_hooks" in sys.modules:
        return
    try:
        from trn_agent_boot.trn_boot import _ntff_profile_via_ctypes

        hook = _ntff_profile_via_ctypes("/opt/axon/libaxon_pjrt.so")
        if hook is None:
            return
        mod = types.ModuleType("antenv.axon_hooks")
        mod.get_axon_ntff_profile_hook = lambda: hook
        mod.set_axon_ntff_profile_hook = lambda h: None
        sys.modules["antenv.axon_hooks"] = mod
        from concourse import bass_utils

        bass_utils.upload_artifacts = lambda tmpdir: f"local:{tmpdir}"
    except Exception:
        pass


_LAST_RESULTS = None


def kernel(x: np.ndarray, coeff: np.ndarray) -> np.ndarray:
    global _LAST_RESULTS
    from concourse import bass_utils

    _maybe_install_profile_shim()

    x = np.ascontiguousarray(np.asarray(x), dtype=np.float32)
    coeff = np.asarray(coeff)
    assert x.shape == (B_FULL, CIN, H, W), x.shape

    w_host = _fold_coeff(coeff)
    btbl = np.zeros((128, NQ), dtype=np.float32)
    for p in range(128):
        for q in range(NQ):
            m = 2 * q + (1 if p >= 64 else 0)
            btbl[p, q] = -float(m + 2)

    nc = _build_bass()

    in_maps = []
    for i in range(N_CORES):
        in_maps.append({
            "x": np.ascontiguousarray(x[i * B_SHARD:(i + 1) * B_SHARD]),
            "w": w_host,
            "btbl": btbl,
        })

    res = bass_utils.run_bass_kernel_spmd(
        nc, in_maps, core_ids=list(range(N_CORES)),
        trace=bool(os.environ.get("KAN_TRACE")),
    )
    _LAST_RESULTS = res

    out = np.concatenate([res.results[i]["out"] for i in range(N_CORES)], axis=0)
    return out.astype(np.float32, copy=False)


# revision 15
# speedup vs baseline: 3.1080x; 3.1080x over previous
"""Trainium2 Bass kernel for the ConvolutionalKAN problem.

Math: the KAN conv
    out[b,o,y,x] = sum_{j,kk,l,m} phi_m(11*inp[b,j,y+kk,x+l]) * coeff[o,j,kk,l,m]
with phi_m the degree-3 B-spline basis on uniform knots linspace(0,1,12).
With u = |11x-(m+2)|:  6*phi_m = relu(2-u)^3 - 4*relu(1-u)^3, so folding
coeff/6 into the weights makes this a VALID 3x3 conv over 64*8 = 512 input
channels, computed here entirely in fp16 (sim rel-err ~9e-4, limit 2e-2).

Basis pipeline (8 passes per q-tile of 2 basis fns x 64 cin, all fp16):
    u   = Abs(11x + b)          ACT
    sb  = Square(1 - u)         ACT     (unclamped; clamp comes from bm)
    am  = min(u-2, 0) = -relu(2-u)   DVE tensor_scalar
    bm  = min(u-1, 0) = -relu(1-u)   Pool tensor_scalar
    sa  = am*am                 DVE
    a3m = sa*am = -relu(2-u)^3  DVE
    b3m = sb*bm = -relu(1-u)^3  DVE
    g   = 4*b3m - a3m = 6*phi   DVE scalar_tensor_tensor

PE ("L-pack", M=128): x-taps l=0,1 are packed as the 128 lhsT columns
(full PE array) accumulating psum_AB[(l,o), y, xs] over raw 64-wide rows
(N=512/bank); l=2 runs half-width chains with two output groups sharing one
PSUM bank via tile_position. Column shifts are resolved by two shifted
post-adds straight out of PSUM (Pool + DVE):
    out[o,y,x] = psAB[o,y,x] + psAB[64+o,y,x+1] + psC[o,y,x+2]
This needs ~24.4 PE cycles/output pixel vs 36 for the M=64 windowed form.

Sharding: data-parallel over batch, 2 images per core on 8 cores.
"""

import os
import sys

import numpy as np

for _p in ("/root/.axon_site/_ro/trn_rl_repo", "/opt/trn_rl_repo"):
    if os.path.isdir(_p) and _p not in sys.path:
        sys.path.append(_p)

B_FULL = 16
N_CORES = 8
B_SHARD = B_FULL // N_CORES
CIN = 64
COUT = 64
H = 64
W = 64
KS = 3
NB = 8
HO = H - KS + 1  # 62
WO = W - KS + 1  # 62
NQ = (CIN * NB) // 128  # 4 contraction tiles of 128
NPIX = H * W

# input-row chunks for the elementwise basis pipeline (aligned so chunk 0
# exactly covers the rows needed by the first two output groups)
CHUNKS = [(0, 18), (18, 16), (34, 16), (50, 14)]
# output row groups (y0, n_rows)
GROUPS = [(0, 8), (8, 8), (16, 8), (24, 8), (32, 8), (40, 8), (48, 8), (56, 6)]
# pairs of groups sharing one C-chain PSUM bank
WAVES = [(0, 1), (2, 3), (4, 5), (6, 7)]

# Engine picks. Measured on HW: tensor_scalar(2-op) and scalar_tensor_tensor
# are microcode-slow (3-13us/pass) -- avoid; ACT activation ~1.05us and plain
# tensor_tensor fp16 ~1.19us per [128,1152] pass are the fast primitives.
SA_ENG = os.environ.get("KAN_SA_ENG", "scalar")   # sa square: scalar|vector
SB_ENG = os.environ.get("KAN_SB_ENG", "gpsimd")   # sb square: gpsimd|vector
# NOTE: GPSIMD/Pool cannot access PSUM, and no instruction may read two
# PSUM operands; shifted l=1 half goes PSUM->SBUF via DMA (or ACT copy).
CP_ENG = os.environ.get("KAN_CP_ENG", "scalar")  # dma_start cannot read PSUM
C2 = 4.0 ** (1.0 / 3.0)


def _fold_coeff(coeff: np.ndarray) -> np.ndarray:
    """coeff [COUT, CIN, KS, KS, NB] -> W_host [128, NQ*KS*192] fp16.

    Column layout per (q, kk) block of 192: [l=0 | l=1] form the 128-wide
    AB pair, [l=2] the 64-wide C block. Partition k = (m - 2q)*64 + cin.
    """
    w6 = coeff.astype(np.float64) / 6.0  # [o, c, kk, l, m]
    arr = np.transpose(w6, (4, 1, 2, 3, 0))  # [m, c, kk, l, o]
    blocks = []
    for q in range(NQ):
        blk = arr[2 * q:2 * q + 2].reshape(128, KS, KS * COUT)  # [k, kk, l*o]
        blocks.append(blk.reshape(128, KS * KS * COUT))
    whost = np.concatenate(blocks, axis=1)  # [128, NQ*576]
    return np.ascontiguousarray(whost, dtype=np.float16)


def _build_bass():
    import concourse.bacc as bacc
    import concourse.mybir as mybir
    import concourse.tile as tile

    f32 = mybir.dt.float32
    f16 = mybir.dt.float16
    AF = mybir.ActivationFunctionType
    alu = mybir.AluOpType

    nc = bacc.Bacc("TRN2", target_bir_lowering=False, debug=False,
                   num_devices=N_CORES)
    x_d = nc.dram_tensor("x", [B_SHARD, CIN, H, W], f32, kind="ExternalInput").ap()
    w_d = nc.dram_tensor("w", [128, NQ * KS * KS * COUT], f16,
                         kind="ExternalInput").ap()
    b_d = nc.dram_tensor("btbl", [128, NQ + 2], f32, kind="ExternalInput").ap()
    out_d = nc.dram_tensor("out", [B_SHARD, COUT, HO, WO], f32,
                           kind="ExternalOutput").ap()

    with tile.TileContext(nc) as tc:
        from contextlib import ExitStack

        with ExitStack() as ctx:
            cpool = ctx.enter_context(tc.tile_pool(name="const", bufs=1))
            wpool = ctx.enter_context(tc.tile_pool(name="w", bufs=1))
            xpool = ctx.enter_context(tc.tile_pool(name="x", bufs=2))
            gpool = ctx.enter_context(tc.tile_pool(name="g", bufs=2 * NQ))
            rpool = ctx.enter_context(tc.tile_pool(name="r", bufs=3))
            opool = ctx.enter_context(tc.tile_pool(name="o", bufs=4))
            pabpool = ctx.enter_context(
                tc.tile_pool(name="pab", bufs=6, space="PSUM"))

            bt = cpool.tile([128, NQ + 2], f32)
            nc.sync.dma_start(bt[:], b_d[:])
            wc = wpool.tile([128, NQ * KS * KS * COUT], f16)
            nc.sync.dma_start(wc[:], w_d[:])

            def eng(name):
                return {"scalar": nc.scalar, "vector": nc.vector,
                        "gpsimd": nc.gpsimd}[name]

            for b in range(B_SHARD):
                xt = xpool.tile([128, NPIX], f32)
                src = x_d[b, :, :, :]
                nc.sync.dma_start(
                    xt[0:64].rearrange("p (r c) -> p r c", c=W), src)
                nc.sync.dma_start(
                    xt[64:128].rearrange("p (r c) -> p r c", c=W), src)

                gts = []
                for q in range(NQ):
                    gts.append(gpool.tile([128, NPIX], f16, tag="g",
                                          name=f"g{b}_{q}"))

                # elementwise basis, chunked by input-row ranges.
                # g = relu(2-u)^3 - (c2*relu(1-u))^3 = 6*phi, all fp16.
                for (r0, nr_in) in CHUNKS:
                    npx = nr_in * W
                    lo, hi = r0 * W, r0 * W + npx
                    for q in range(NQ):
                        u = rpool.tile([128, 18 * W], f16, tag="u")
                        a = rpool.tile([128, 18 * W], f16, tag="a")
                        bb = rpool.tile([128, 18 * W], f16, tag="bb")
                        sa = rpool.tile([128, 18 * W], f16, tag="sa")
                        sb = rpool.tile([128, 18 * W], f16, tag="sb")
                        a3 = rpool.tile([128, 18 * W], f16, tag="a3")
                        b3 = rpool.tile([128, 18 * W], f16, tag="b3")
                        nc.scalar.activation(u[:, :npx], xt[:, lo:hi],
                                             AF.Abs, bias=bt[:, q:q + 1],
                                             scale=11.0)
                        nc.scalar.activation(a[:, :npx], u[:, :npx],
                                             AF.Relu, bias=bt[:, NQ:NQ + 1], scale=-1.0)
                        nc.scalar.activation(bb[:, :npx], u[:, :npx],
                                             AF.Relu, bias=bt[:, NQ + 1:NQ + 2], scale=-C2)
                        if SA_ENG == "scalar":
                            nc.scalar.activation(sa[:, :npx], u[:, :npx],
                                                 AF.Square, bias=bt[:, NQ:NQ + 1],
                                                 scale=-1.0)
                        else:
                            nc.vector.tensor_mul(sa[:, :npx], a[:, :npx],
                                                 a[:, :npx])
                        eng(SB_ENG).tensor_mul(sb[:, :npx], bb[:, :npx],
                                               bb[:, :npx])
                        nc.vector.tensor_mul(a3[:, :npx], sa[:, :npx],
                                             a[:, :npx])
                        nc.vector.tensor_mul(b3[:, :npx], sb[:, :npx],
                                             bb[:, :npx])
                        nc.vector.tensor_sub(gts[q][:, lo:hi], a3[:, :npx],
                                             b3[:, :npx])

                gvs = [g[:].rearrange("p (r c) -> p r c", c=W) for g in gts]

                # matmuls: per wave of 2 groups, one PSUM bank per group.
                # Bank layout: partitions 0:64 accumulate l=0 (raw rows) and
                # l=2 (rhs column-shifted by 2, M=64); partitions 64:128
                # accumulate l=1 (raw rows, paired in the same lhsT as l=0).
                n_mm = NQ * KS
                for (ga, gb) in WAVES:
                    wave = (ga, gb)
                    psab = {g: pabpool.tile([128, 8, W], f32, tag="psab",
                                            name=f"psab{g}")
                            for g in wave}
                    i_mm = 0
                    for q in range(NQ):
                        for kk in range(KS):
                            base = q * (KS * KS * COUT) + kk * (KS * COUT)
                            wab = wc[:, base:base + 2 * COUT]
                            wcc = wc[:, base + 2 * COUT:base + 3 * COUT]
                            first = i_mm == 0
                            last = i_mm == n_mm - 1
                            for grp in wave:
                                y0, nr = GROUPS[grp]
                                rhs = gvs[q][:, y0 + kk:y0 + kk + nr, :]
                                rhs2 = gvs[q][:, y0 + kk:y0 + kk + nr, 2:W]
                                if first:
                                    nc.tensor.matmul(psab[grp][:, :nr, :],
                                                     wab, rhs, start=True,
                                                     stop=False)
                                    nc.tensor.matmul(
                                        psab[grp][0:64, :nr, 0:W - 2], wcc,
                                        rhs2, start=False, stop=False)
                                else:
                                    nc.tensor.matmul(
                                        psab[grp][0:64, :nr, 0:W - 2], wcc,
                                        rhs2, start=False, stop=last)
                                    nc.tensor.matmul(psab[grp][:, :nr, :],
                                                     wab, rhs, start=False,
                                                     stop=last)
                            i_mm += 1
                    for grp in wave:
                        y0, nr = GROUPS[grp]
                        t = opool.tile([64, 8, WO], f32, tag="t")
                        if CP_ENG == "dma":
                            nc.sync.dma_start(
                                t[:, :nr, :], psab[grp][64:128, :nr, 1:1 + WO])
                        else:
                            nc.scalar.activation(
                                t[:, :nr, :], psab[grp][64:128, :nr, 1:1 + WO],
                                AF.Identity, bias=0.0, scale=1.0)
                        ot = opool.tile([64, 8, WO], f32, tag="ot")
                        nc.vector.tensor_tensor(
                            ot[:, :nr, :], t[:, :nr, :],
                            psab[grp][0:64, :nr, 0:WO], alu.add)
                        nc.sync.dma_start(out_d[b, :, y0:y0 + nr, :],
                                          ot[:, :nr, :])

    nc.compile()
    return nc


def _maybe_install_profile_shim():
    """Allow trace=True/BASS_TRACE under axon even though this image lacks
    antenv.axon_hooks; degrade silently if anything is missing."""
    import types

    if "antenv.axon_hooks" in sys.modules:
        return
    try:
        from trn_agent_boot.trn_boot import _ntff_profile_via_ctypes

        hook = _ntff_profile_via_ctypes("/opt/axon/libaxon_pjrt.so")
        if hook is None:
            return
        mod = types.ModuleType("antenv.axon_hooks")
        mod.get_axon_ntff_profile_hook = lambda: hook
        mod.set_axon_ntff_profile_hook = lambda h: None
        sys.modules["antenv.axon_hooks"] = mod
        from concourse import bass_utils

        bass_utils.upload_artifacts = lambda tmpdir: f"local:{tmpdir}"
    except Exception:
        pass


_LAST_RESULTS = None


def kernel(x: np.ndarray, coeff: np.ndarray) -> np.ndarray:
    global _LAST_RESULTS
    from concourse import bass_utils

    _maybe_install_profile_shim()

    x = np.ascontiguousarray(np.asarray(x), dtype=np.float32)
    coeff = np.asarray(coeff)
    assert x.shape == (B_FULL, CIN, H, W), x.shape

    w_host = _fold_coeff(coeff)
    btbl = np.zeros((128, NQ + 2), dtype=np.float32)
    for p in range(128):
        for q in range(NQ):
            m = 2 * q + (1 if p >= 64 else 0)
            btbl[p, q] = -float(m + 2)
    btbl[:, NQ] = 2.0
    btbl[:, NQ + 1] = C2

    nc = _build_bass()

    in_maps = []
    for i in range(N_CORES):
        in_maps.append({
            "x": np.ascontiguousarray(x[i * B_SHARD:(i + 1) * B_SHARD]),
            "w": w_host,
            "btbl": btbl,
        })

    res = bass_utils.run_bass_kernel_spmd(
        nc, in_maps, core_ids=list(range(N_CORES)),
        trace=bool(os.environ.get("KAN_TRACE")),
    )
    _LAST_RESULTS = res

    out = np.concatenate([res.results[i]["out"] for i in range(N_CORES)], axis=0)
    return out.astype(np.float32, copy=False)
